# revision 1
# baseline (speedup 1.0000x reference)
"""Compact Bilinear Pooling (count-sketch + FFT + sum-pool) on 8 trn2 cores.

Math: for each spatial position n, the count-sketch followed by rFFT is
    F[n, k] = sum_c x[n, c] * s[c] * exp(-2*pi*i * k * h[c] / 8192)
i.e. a dense [N, C] @ [C, K] complex matmul with an input-dependent DFT
matrix E[c, k] = s[c] * W^(k*h[c]).  The circular-convolution spectrum is
P[n, k] = F1[n, k] * F2[n, k]; summing P over the positions of each sample
(linearity of the IFFT) gives the pooled spectrum, one small irfft per
sample recovers the pooled feature map, then signed-sqrt + L2 normalize.

Sharding: frequency bins k = 0..4095 are split 512-per-core (rFFT needs
only k <= 4096; the single Nyquist bin k=4096 is done on the host, it is
1/4097 of the work).  Each core computes, for its k-slice:
  - F-components as fp16 matmuls  E[c,k]^T @ xT[c,n]  ->  PSUM [k, n]
  - ScalarE evacuates PSUM -> SBUF; the DVE computes the four real products
    of the complex multiply fused with the per-sample reduction
    (scalar_tensor_tensor with accum_out) -> acc[k, b]
The host assembles the complex spectrum, does the [16, 4097] irfft and the
final normalization.
"""

import numpy as np
import ml_dtypes

PROJ = 8192
B, C, H, W = 16, 512, 14, 14
HWN = H * W           # 196 positions per sample
NTOT = B * HWN        # 3136
NCORES = 8
KDEV = 4096           # k bins computed on device (0..4095)
KS = KDEV // NCORES   # 512 per core
NT = 2 * HWN          # 392: two samples per n-tile
NNT = NTOT // NT      # 8 n-tiles
NKT = KS // 128       # 4 k-tiles of 128 per core
THRESH = 1e-8
L2_EPS = 1e-12

MM_DTYPE = "fp16"     # "fp16": FWL-speed weight loads, ~7e-4 end-to-end err
                      # "f32r": tf32-like, ~3e-4 err but 2x slower LDWEIGHTS
                      # "bf16": fastest-equal, ~5e-3 err
PROD_F32 = True      # fp32 DVE products (1x mode) instead of bf16 (2x mode)
TRACE = False         # set by test.py to collect HW timing
LAST_RESULT = {}      # exec_time_ns etc. for test.py

_NC_CACHE = {}


def _install_ntff_hook():
    """The container's antenv stub lacks axon_hooks, so the boot-time NTFF
    profile hook install silently degraded.  Recreate it: a tiny module
    backed by ctypes calls into libaxon_pjrt.so (same mechanism as
    trn_agent_boot.trn_boot)."""
    import sys, types
    if "antenv.axon_hooks" in sys.modules:
        return
    try:
        from trn_agent_boot.trn_boot import _ntff_profile_via_ctypes
        hook = _ntff_profile_via_ctypes("/opt/axon/libaxon_pjrt.so")
    except Exception:
        hook = None
    mod = types.ModuleType("antenv.axon_hooks")
    _state = {"hook": hook}
    mod.get_axon_ntff_profile_hook = lambda: _state["hook"]
    mod.set_axon_ntff_profile_hook = lambda h: _state.__setitem__("hook", h)
    sys.modules["antenv.axon_hooks"] = mod
    try:
        import antenv
        antenv.axon_hooks = mod
    except Exception:
        pass


def _split_multiwaits(nc, maxw=1):
    """This container's walrus codegen rejects instructions carrying more
    than one sem wait ("Too many sync wait commands").  Hoist excess waits
    onto same-engine NoOps inserted immediately before the instruction —
    semantically identical (the engine sequencer blocks either way)."""
    import bass_rust
    import concourse.mybir as mybir

    for f in nc.m.functions:
        for bb in f.blocks:
            il = bb.instructions
            new = []
            changed = False
            for inst in il:
                si = inst.sync_info
                waits = list(si.on_wait) if si is not None else []
                if len(waits) > maxw:
                    keep = waits[-maxw:]
                    for w in waits[:-maxw]:
                        nop = mybir.InstNoOp(
                            name=nc.get_next_instruction_name(),
                            engine=inst.engine,
                            sync_info=bass_rust.SyncInfo(
                                on_wait=[w], on_update=[]
                            ),
                            bass_nofuse=True,
                        )
                        nc.register_instruction(nop)
                        new.append(nop)
                    inst.sync_info = bass_rust.SyncInfo(
                        on_wait=keep, on_update=list(si.on_update)
                    )
                    changed = True
                new.append(inst)
            if changed:
                bb.instructions = new


def _build_nc():
    import concourse.bass as bass
    import concourse.mybir as mybir
    import concourse.tile as tile
    from concourse.vector_clock import ScopedClock

    class TrimTC(tile.TileContext):
        # Stock tail: drain + barrier + sem clears + barrier (~10us).
        # The sem clears are required for NEFF re-execution, but they can
        # ride behind the first barrier without a trailing second barrier:
        # nothing after them reads the sems, and the next execution's
        # preamble re-syncs the engines.
        def _drain_and_barrier(self, tick_clock, wait_clock):
            drain_inst = self.nc.sync.drain()
            wait_clock.add_sem_waits(
                drain_inst.ins, ScopedClock({None: tick_clock.global_clock})
            )
            popped = self.nc._tile_sem_poison_stack.pop()
            assert popped is self._sem_poison
            # no barrier / sem clears: the SP drain already waits on the
            # final DMA sems, NRT's own completion protocol syncs engines,
            # and the execution preamble re-initializes semaphores
            # (verified by back-to-back runs).

    bf16 = mybir.dt.bfloat16
    f32 = mybir.dt.float32
    mult = mybir.AluOpType.mult

    mmdt = {"fp16": mybir.dt.float16, "f32r": mybir.dt.float32r, "bf16": bf16}[MM_DTYPE]
    proddt = f32 if PROD_F32 else bf16

    nc = bass.Bass("TRN2", target_bir_lowering=False, debug=False)
    # xT host layout: [nt, ct, 128c, 392n]; e host layout: [kt, ct, 128c, comp, 128k]
    xT_d = nc.dram_tensor("xT", [NNT, 4, 128, NT], mmdt, kind="ExternalInput")
    e_d = nc.dram_tensor("e", [NKT, 4, 128, 4, 128], mmdt, kind="ExternalInput")
    out_d = nc.dram_tensor("out", [2, NKT, 128, B], f32, kind="ExternalOutput")

    with TrimTC(nc) as tc:
        with (
            tc.tile_pool(name="const", bufs=1) as const,
            tc.tile_pool(name="fpsum", bufs=4, space="PSUM") as fpsum,
            tc.tile_pool(name="fbsb", bufs=3) as fbsb,
            tc.tile_pool(name="scratch", bufs=4) as scratch,
            tc.tile_pool(name="outp", bufs=2) as outp,
        ):
            # One big contiguous DMA per (kt, ct) e-block and per nt x-block,
            # emitted in consumption order so the first matmul chain starts
            # after just 2 transfers.
            es = [
                [const.tile([128, 512], mmdt, name=f"e_{kt}_{ct}") for ct in range(4)]
                for kt in range(NKT)
            ]
            xs = [const.tile([128, 4, NT], mmdt, name=f"x_{nt}") for nt in range(NNT)]

            def dma_e(kt, eng):
                for ct in range(4):
                    eng.dma_start(es[kt][ct][:], e_d[kt, ct])

            def dma_x(nt):
                nc.sync.dma_start(
                    xs[nt][:], xT_d[nt].rearrange("ct c n -> c ct n")
                )

            # first set split across both HWDGE queues so the first matmul
            # chain's inputs land in parallel; everything else on SP.
            dma_x(0)
            dma_e(0, nc.scalar)
            for nt in range(1, NNT):
                dma_x(nt)
            for kt in range(1, NKT):
                dma_e(kt, nc.sync)

            # Warm the PE clock gate (HAM) with throwaway matmuls on
            # never-written SBUF garbage while the first input DMAs land.
            wsrc = const.tile([128, 128], bf16, name="warm_src")
            wrhs = const.tile([128, 64], bf16, name="warm_rhs")
            nc.gpsimd.memset(wsrc[:], 0.0)
            nc.gpsimd.memset(wrhs[:], 0.0)
            wps = fpsum.tile([128, 2 * 512], f32, name="F", tag="F")
            for _ in range(50):
                nc.tensor.matmul(wps[:, :64], wsrc[:], wrhs[:], start=True, stop=True)

            acc = [
                [const.tile([128, B], f32, name=f"acc_{kt}_{a}") for a in range(4)]
                for kt in range(NKT)
            ]

            # products: (F1r*F2r, F1i*F2i, F1r*F2i, F1i*F2r) -> (ac, bd, ad, bc)
            prods = [(0, 2), (1, 3), (0, 3), (1, 2)]

            BANKW = 512  # fp32 elements per PSUM bank
            # n-tiles processed in pairs: each LDWEIGHTS feeds two matmuls
            # (f32r weight loads are 2x slower than bf16 and would otherwise
            # be exposed).  Per component, a 2-bank PSUM tile holds n-tile A
            # at cols [0:392) and n-tile B at cols [512:904); ScalarE
            # evacuates per component so banks recycle at comp granularity.
            for kt in range(NKT):
                for p in range(NNT // 2):
                    ntA, ntB = 2 * p, 2 * p + 1
                    for comp in range(4):
                        F = fpsum.tile([128, 2 * BANKW], f32, name="F", tag="F")
                        for ct in range(4):
                            lhs = es[kt][ct][:, comp * 128 : (comp + 1) * 128]
                            nc.tensor.matmul(
                                F[:, 0:NT], lhs, xs[ntA][:, ct],
                                start=(ct == 0), stop=(ct == 3),
                            )
                            nc.tensor.matmul(
                                F[:, BANKW : BANKW + NT], lhs, xs[ntB][:, ct],
                                start=(ct == 0), stop=(ct == 3),
                            )
                        Fb = fbsb.tile(
                            [128, BANKW + NT], proddt, name="Fb", tag=f"Fb{comp}"
                        )
                        nc.scalar.copy(Fb[:], F[:, 0 : BANKW + NT])
                        if comp == 0:
                            Fbs = [Fb]
                        else:
                            Fbs.append(Fb)
                    for s in range(4):
                        b = 4 * p + s
                        off = (s // 2) * BANKW + (s % 2) * HWN
                        psl = slice(off, off + HWN)
                        for a, (i, j) in enumerate(prods):
                            sc = scratch.tile([128, HWN], proddt, name="sc", tag="sc")
                            nc.vector.scalar_tensor_tensor(
                                out=sc[:],
                                in0=Fbs[i][:, psl],
                                scalar=1.0,
                                in1=Fbs[j][:, psl],
                                op0=mult,
                                op1=mult,
                                accum_out=acc[kt][a][:, b : b + 1],
                            )

                # Pr/Pi combines on the idle GpSimd engine, off the DVE.
                pr = outp.tile([128, B], f32, name="pr", tag=f"o{kt}")
                nc.gpsimd.tensor_sub(pr[:], acc[kt][0][:], acc[kt][1][:])
                nc.sync.dma_start(out_d[0, kt], pr[:])
                pi = outp.tile([128, B], f32, name="pi", tag=f"o{kt}2")
                nc.gpsimd.tensor_add(pi[:], acc[kt][2][:], acc[kt][3][:])
                nc.sync.dma_start(out_d[1, kt], pi[:])

    _split_multiwaits(nc)
    return nc


def _get_nc():
    if "nc" not in _NC_CACHE:
        _NC_CACHE["nc"] = _build_nc()
    return _NC_CACHE["nc"]


def kernel(x, s1, s2, h1, h2):
    if TRACE:
        _install_ntff_hook()
    from concourse.bass_utils import run_bass_kernel_spmd

    x = np.asarray(x, dtype=np.float32)
    s1 = np.asarray(s1, dtype=np.float64)
    s2 = np.asarray(s2, dtype=np.float64)
    h1 = np.asarray(h1).astype(np.int64)
    h2 = np.asarray(h2).astype(np.int64)

    # x [B, C, H, W] -> xT [C, B*H*W] (natural: transpose batch to columns)
    xT = x.transpose(1, 0, 2, 3).reshape(C, NTOT)
    # device layout [nt, ct, 128c, 392n]
    _mmdt = {"fp16": np.float16, "f32r": np.float32, "bf16": ml_dtypes.bfloat16}[MM_DTYPE]
    xT_dev = np.ascontiguousarray(
        xT.astype(_mmdt)
        .reshape(4, 128, NNT, NT)
        .transpose(2, 0, 1, 3)
    )

    # DFT-of-scatter matrices, E[c, k] = s[c] * W^(k*h[c]), W = exp(-2pi i/PROJ)
    j = np.arange(PROJ)
    cos_t = np.cos(2 * np.pi * j / PROJ)
    sin_t = np.sin(2 * np.pi * j / PROJ)
    k = np.arange(KDEV)
    idx1 = (h1[:, None] * k[None, :]) % PROJ
    idx2 = (h2[:, None] * k[None, :]) % PROJ
    # components: 0=F1r, 1=F1i, 2=F2r, 3=F2i
    E = np.empty((4, C, KDEV), dtype=_mmdt)
    E[0] = (s1[:, None] * cos_t[idx1]).astype(_mmdt)
    E[1] = (-s1[:, None] * sin_t[idx1]).astype(_mmdt)
    E[2] = (s2[:, None] * cos_t[idx2]).astype(_mmdt)
    E[3] = (-s2[:, None] * sin_t[idx2]).astype(_mmdt)

    nc = _get_nc()
    # device e layout: [kt, ct, 128c, comp, 128k], k_local = kt*128 + kk
    in_maps = [
        {
            "xT": xT_dev,
            "e": np.ascontiguousarray(
                E[:, :, m * KS : (m + 1) * KS]       # [comp, c, 512k]
                .reshape(4, 4, 128, NKT, 128)        # [comp, ct, c128, kt, kk]
                .transpose(3, 1, 2, 0, 4)            # [kt, ct, c128, comp, kk]
            ),
        }
        for m in range(NCORES)
    ]
    res = run_bass_kernel_spmd(
        nc, in_maps, core_ids=list(range(NCORES)), trace=TRACE
    )
    LAST_RESULT["exec_time_ns"] = res.exec_time_ns
    LAST_RESULT["mean_exec_time_ns"] = res.mean_exec_time_ns
    LAST_RESULT["res"] = res

    # assemble spectrum: out [2, NKT, 128, B] per core, k = m*KS + kt*128 + kk
    spec = np.empty((B, KDEV + 1), dtype=np.complex128)
    for m in range(NCORES):
        o = res.results[m]["out"].astype(np.float64)  # [2, NKT, 128, B]
        pk = (o[0] + 1j * o[1]).reshape(KS, B)  # [k_local, B]
        spec[:, m * KS : (m + 1) * KS] = pk.T

    # Nyquist bin k=4096 on host: W^(4096*h) = (-1)^h (real)
    xT64 = xT.astype(np.float64)
    f1ny = ((s1 * np.where(h1 % 2 == 0, 1.0, -1.0)) @ xT64)  # [NTOT]
    f2ny = ((s2 * np.where(h2 % 2 == 0, 1.0, -1.0)) @ xT64)
    spec[:, KDEV] = (f1ny * f2ny).reshape(B, HWN).sum(axis=1)

    y = np.fft.irfft(spec, n=PROJ, axis=1)  # [B, PROJ]
    y = np.sign(y) * np.sqrt(np.abs(y) + THRESH)
    nrm = np.linalg.norm(y, axis=1, keepdims=True)
    y = y / np.maximum(nrm, L2_EPS)
    return y.astype(np.float32)



# revision 2
# speedup vs baseline: 4.9537x; 4.9537x over previous
"""Compact Bilinear Pooling (count-sketch + FFT + sum-pool) on 8 trn2 cores.

Math: the pooled output of sample b depends on x only through the Gram
matrix  G_b[c1, c2] = sum_n x[b, c1, n] * x[b, c2, n]  (n = spatial pos):

    y_b[k] = sum_{c1, c2} s1[c1] s2[c2] G_b[c1, c2]
                          * [(h1[c1] + h2[c2]) mod 8192 == k]

because the per-position circular convolution of the two count sketches,
summed over positions, is the bilinear form above (expand the sketches:
each channel pair (c1, c2) lands in bin (h1[c1]+h2[c2]) mod P with weight
s1 s2 x[n,c1] x[n,c2]; the position sum yields G_b).  This removes the
FFTs entirely: the device computes the 16 Gram matrices (a [196, 512]^T
@ [196, 512] matmul per sample, 103 MFLOP), and the host applies the
fixed 262144-pair scatter (np.bincount), signed sqrt and L2 normalize.

Sharding: pure data parallel, 2 samples per core.  Per core the device
reads xᵀ [2, 196, 512] fp16 (0.4 MB), runs 16 accumulating matmuls into
PSUM (G blocks [128c1, 512c2], contraction over positions 196 = 128+68),
ScalarE evacuates PSUM -> SBUF fp16, one 512 KB DMA per sample writes G
back.  ~1.4 MB of HBM traffic per core: memory-bound.
"""

import numpy as np

PROJ = 8192
B, C, H, W = 16, 512, 14, 14
HWN = H * W           # 196 positions per sample
NCORES = 8
SPC = B // NCORES     # 2 samples per core
NCH0 = 128            # position-chunk sizes (contraction dim)
NCH1 = HWN - NCH0     # 68
THRESH = 1e-8
L2_EPS = 1e-12

TRACE = False         # set by test.py to collect HW timing
LAST_RESULT = {}      # exec_time_ns etc. for test.py

_NC_CACHE = {}


def _install_ntff_hook():
    """The container's antenv stub lacks axon_hooks, so the boot-time NTFF
    profile hook install silently degraded.  Recreate it: a tiny module
    backed by ctypes calls into libaxon_pjrt.so (same mechanism as
    trn_agent_boot.trn_boot)."""
    import sys, types
    if "antenv.axon_hooks" in sys.modules:
        return
    try:
        from trn_agent_boot.trn_boot import _ntff_profile_via_ctypes
        hook = _ntff_profile_via_ctypes("/opt/axon/libaxon_pjrt.so")
    except Exception:
        hook = None
    mod = types.ModuleType("antenv.axon_hooks")
    _state = {"hook": hook}
    mod.get_axon_ntff_profile_hook = lambda: _state["hook"]
    mod.set_axon_ntff_profile_hook = lambda h: _state.__setitem__("hook", h)
    sys.modules["antenv.axon_hooks"] = mod
    try:
        import antenv
        antenv.axon_hooks = mod
    except Exception:
        pass


def _split_multiwaits(nc, maxw=1):
    """This container's walrus codegen rejects instructions carrying more
    than one sem wait ("Too many sync wait commands").  Hoist excess waits
    onto same-engine NoOps inserted immediately before the instruction —
    semantically identical (the engine sequencer blocks either way)."""
    import bass_rust
    import concourse.mybir as mybir

    for f in nc.m.functions:
        for bb in f.blocks:
            il = bb.instructions
            new = []
            changed = False
            for inst in il:
                si = inst.sync_info
                waits = list(si.on_wait) if si is not None else []
                if len(waits) > maxw:
                    keep = waits[-maxw:]
                    for w in waits[:-maxw]:
                        nop = mybir.InstNoOp(
                            name=nc.get_next_instruction_name(),
                            engine=inst.engine,
                            sync_info=bass_rust.SyncInfo(
                                on_wait=[w], on_update=[]
                            ),
                            bass_nofuse=True,
                        )
                        nc.register_instruction(nop)
                        new.append(nop)
                    inst.sync_info = bass_rust.SyncInfo(
                        on_wait=keep, on_update=list(si.on_update)
                    )
                    changed = True
                new.append(inst)
            if changed:
                bb.instructions = new


def _build_nc():
    import concourse.bass as bass
    import concourse.mybir as mybir
    import concourse.tile as tile
    from concourse.vector_clock import ScopedClock

    class TrimTC(tile.TileContext):
        # Stock tail: drain + barrier + sem clears + barrier (~10us).
        # The sem clears are required for NEFF re-execution, but they can
        # ride behind the first barrier without a trailing second barrier:
        # nothing after them reads the sems, and the next execution's
        # preamble re-syncs the engines.
        def _drain_and_barrier(self, tick_clock, wait_clock):
            drain_inst = self.nc.sync.drain()
            wait_clock.add_sem_waits(
                drain_inst.ins, ScopedClock({None: tick_clock.global_clock})
            )
            popped = self.nc._tile_sem_poison_stack.pop()
            assert popped is self._sem_poison
            # no barrier / sem clears: the SP drain already waits on the
            # final DMA sems, NRT's own completion protocol syncs engines,
            # and the execution preamble re-initializes semaphores
            # (verified by back-to-back runs).

    bf16 = mybir.dt.bfloat16
    f16 = mybir.dt.float16
    f32 = mybir.dt.float32

    nc = bass.Bass("TRN2", target_bir_lowering=False, debug=False)
    # x^T per sample: positions on the contraction (partition) axis
    xd = nc.dram_tensor("xin", [SPC, HWN, C], f16, kind="ExternalInput")
    # G blocks: out[s, p, 4*512] with G[s, 128*i + p, c2] = out[s, p, 512*i + c2]
    out_d = nc.dram_tensor("out", [SPC, 128, 4 * C], f16, kind="ExternalOutput")

    with TrimTC(nc) as tc:
        with (
            tc.tile_pool(name="const", bufs=1) as const,
            tc.tile_pool(name="gpsum", bufs=8, space="PSUM") as gpsum,
            tc.tile_pool(name="gout", bufs=2) as goutp,
        ):
            xt = []
            for s in range(SPC):
                t0 = const.tile([NCH0, C], f16, name=f"x{s}_0")
                t1 = const.tile([NCH1, C], f16, name=f"x{s}_1")
                xt.append((t0, t1))
            # input DMAs split across the two HWDGE rings
            nc.sync.dma_start(xt[0][0][:], xd[0, 0:NCH0])
            nc.scalar.dma_start(xt[0][1][:], xd[0, NCH0:HWN])
            nc.sync.dma_start(xt[1][0][:], xd[1, 0:NCH0])
            nc.scalar.dma_start(xt[1][1][:], xd[1, NCH0:HWN])

            # Warm the PE clock gate (HAM) with throwaway matmuls on
            # never-written SBUF garbage while the input DMAs land.
            wsrc = const.tile([128, 128], bf16, name="warm_src")
            wrhs = const.tile([128, 64], bf16, name="warm_rhs")
            nc.gpsimd.memset(wsrc[:], 0.0)
            nc.gpsimd.memset(wrhs[:], 0.0)
            wps = gpsum.tile([128, C], f32, name="ps", tag="ps")
            for _ in range(40):
                nc.tensor.matmul(wps[:, :64], wsrc[:], wrhs[:], start=True, stop=True)

            for s in range(SPC):
                go = goutp.tile([128, 4 * C], f16, name=f"g{s}", tag="go")
                t0, t1 = xt[s]
                for i in range(4):
                    ps = gpsum.tile([128, C], f32, name="ps", tag="ps")
                    nc.tensor.matmul(
                        ps[:], t0[:, 128 * i : 128 * (i + 1)], t0[:],
                        start=True, stop=False,
                    )
                    nc.tensor.matmul(
                        ps[:], t1[:, 128 * i : 128 * (i + 1)], t1[:],
                        start=False, stop=True,
                    )
                    nc.scalar.copy(go[:, C * i : C * (i + 1)], ps[:])
                # one 512 KB store per sample, alternating HWDGE rings
                eng = nc.scalar if s == 0 else nc.sync
                eng.dma_start(out_d[s], go[:])

    _split_multiwaits(nc)
    return nc


def _get_nc():
    if "nc" not in _NC_CACHE:
        _NC_CACHE["nc"] = _build_nc()
    return _NC_CACHE["nc"]


def kernel(x, s1, s2, h1, h2):
    if TRACE:
        _install_ntff_hook()
    from concourse.bass_utils import run_bass_kernel_spmd

    x = np.asarray(x, dtype=np.float32)
    s1 = np.asarray(s1, dtype=np.float64)
    s2 = np.asarray(s2, dtype=np.float64)
    h1 = np.asarray(h1).astype(np.int64)
    h2 = np.asarray(h2).astype(np.int64)

    # [B, C, H, W] -> [B, HW, C] fp16 (positions on the contraction axis)
    xt = np.ascontiguousarray(
        x.reshape(B, C, HWN).transpose(0, 2, 1)
    ).astype(np.float16)

    nc = _get_nc()
    in_maps = [{"xin": xt[SPC * m : SPC * (m + 1)]} for m in range(NCORES)]
    res = run_bass_kernel_spmd(
        nc, in_maps, core_ids=list(range(NCORES)), trace=TRACE
    )
    LAST_RESULT["exec_time_ns"] = res.exec_time_ns
    LAST_RESULT["mean_exec_time_ns"] = res.mean_exec_time_ns
    LAST_RESULT["res"] = res

    # reassemble G [B, C, C] from the per-core block layout
    G = np.empty((B, C, C), dtype=np.float64)
    for m in range(NCORES):
        o = res.results[m]["out"].astype(np.float64)  # [SPC, 128, 2048]
        G[SPC * m : SPC * (m + 1)] = (
            o.reshape(SPC, 128, 4, C).transpose(0, 2, 1, 3).reshape(SPC, C, C)
        )

    # fixed hash-pair scatter: pair (c1, c2) -> bin (h1[c1]+h2[c2]) mod P
    bins = ((h1[:, None] + h2[None, :]) % PROJ).ravel()
    sw = np.outer(s1, s2).ravel()
    y = np.empty((B, PROJ), dtype=np.float64)
    for b in range(B):
        y[b] = np.bincount(bins, weights=sw * G[b].ravel(), minlength=PROJ)

    y = np.sign(y) * np.sqrt(np.abs(y) + THRESH)
    nrm = np.linalg.norm(y, axis=1, keepdims=True)
    y = y / np.maximum(nrm, L2_EPS)
    return y.astype(np.float32)


# revision 4
# speedup vs baseline: 5.2630x; 1.0624x over previous
"""Compact Bilinear Pooling (count-sketch + FFT + sum-pool) on 8 trn2 cores.

Math: the pooled output of sample b depends on x only through the Gram
matrix  G_b[c1, c2] = sum_n x[b, c1, n] * x[b, c2, n]  (n = spatial pos):

    y_b[k] = sum_{c1, c2} s1[c1] s2[c2] G_b[c1, c2]
                          * [(h1[c1] + h2[c2]) mod 8192 == k]

because the per-position circular convolution of the two count sketches,
summed over positions, is the bilinear form above (expand the sketches:
each channel pair (c1, c2) lands in bin (h1[c1]+h2[c2]) mod P with weight
s1 s2 x[n,c1] x[n,c2]; the position sum yields G_b).  This removes the
FFTs entirely: the device computes the 16 Gram matrices (a [196, 512]^T
@ [196, 512] matmul per sample, 103 MFLOP), and the host applies the
fixed 262144-pair scatter (np.bincount), signed sqrt and L2 normalize.

Sharding: pure data parallel, 2 samples per core.  G is symmetric, so
only the 10 upper-triangle [128, 128] blocks are computed and stored
(rows 128i, cols 128i..512, widths 512/384/256/128).  Contraction over
positions is split 196 = 128 + 68; the four start-matmuls of a sample
run back-to-back so only the first 128-position chunk gates the PE.
PSUM is evacuated alternately by ScalarE and VectorE (fp32 -> fp16) and
each block DMAs out as soon as it is ready, smallest block last to
minimize the completion tail.  ~1 MB HBM traffic per core: memory-bound.
"""

import numpy as np

PROJ = 8192
B, C, H, W = 16, 512, 14, 14
HWN = H * W           # 196 positions per sample
NCORES = 8
SPC = B // NCORES     # 2 samples per core
NCH0 = 128            # position-chunk sizes (contraction dim)
NCH1 = HWN - NCH0     # 68
WIDTHS = [C - 128 * i for i in range(4)]  # triangle block widths
THRESH = 1e-8
L2_EPS = 1e-12

TRACE = False         # set by test.py to collect HW timing
LAST_RESULT = {}      # exec_time_ns etc. for test.py

_NC_CACHE = {}


def _install_ntff_hook():
    """The container's antenv stub lacks axon_hooks, so the boot-time NTFF
    profile hook install silently degraded.  Recreate it: a tiny module
    backed by ctypes calls into libaxon_pjrt.so (same mechanism as
    trn_agent_boot.trn_boot)."""
    import sys, types
    if "antenv.axon_hooks" in sys.modules:
        return
    try:
        from trn_agent_boot.trn_boot import _ntff_profile_via_ctypes
        hook = _ntff_profile_via_ctypes("/opt/axon/libaxon_pjrt.so")
    except Exception:
        hook = None
    mod = types.ModuleType("antenv.axon_hooks")
    _state = {"hook": hook}
    mod.get_axon_ntff_profile_hook = lambda: _state["hook"]
    mod.set_axon_ntff_profile_hook = lambda h: _state.__setitem__("hook", h)
    sys.modules["antenv.axon_hooks"] = mod
    try:
        import antenv
        antenv.axon_hooks = mod
    except Exception:
        pass


def _split_multiwaits(nc, maxw=1):
    """This container's walrus codegen rejects instructions carrying more
    than one sem wait ("Too many sync wait commands").  Hoist excess waits
    onto same-engine NoOps inserted immediately before the instruction —
    semantically identical (the engine sequencer blocks either way)."""
    import bass_rust
    import concourse.mybir as mybir

    for f in nc.m.functions:
        for bb in f.blocks:
            il = bb.instructions
            new = []
            changed = False
            for inst in il:
                si = inst.sync_info
                waits = list(si.on_wait) if si is not None else []
                if len(waits) > maxw:
                    keep = waits[-maxw:]
                    for w in waits[:-maxw]:
                        nop = mybir.InstNoOp(
                            name=nc.get_next_instruction_name(),
                            engine=inst.engine,
                            sync_info=bass_rust.SyncInfo(
                                on_wait=[w], on_update=[]
                            ),
                            bass_nofuse=True,
                        )
                        nc.register_instruction(nop)
                        new.append(nop)
                    inst.sync_info = bass_rust.SyncInfo(
                        on_wait=keep, on_update=list(si.on_update)
                    )
                    changed = True
                new.append(inst)
            if changed:
                bb.instructions = new


def _build_nc():
    import concourse.bass as bass
    import concourse.mybir as mybir
    import concourse.tile as tile
    from concourse.vector_clock import ScopedClock

    class TrimTC(tile.TileContext):
        # Stock tail: drain + barrier + sem clears + barrier (~10us).
        # The sem clears are required for NEFF re-execution, but they can
        # ride behind the first barrier without a trailing second barrier:
        # nothing after them reads the sems, and the next execution's
        # preamble re-syncs the engines.
        def _drain_and_barrier(self, tick_clock, wait_clock):
            drain_inst = self.nc.sync.drain()
            wait_clock.add_sem_waits(
                drain_inst.ins, ScopedClock({None: tick_clock.global_clock})
            )
            popped = self.nc._tile_sem_poison_stack.pop()
            assert popped is self._sem_poison
            # no barrier / sem clears: the SP drain already waits on the
            # final DMA sems, NRT's own completion protocol syncs engines,
            # and the execution preamble re-initializes semaphores
            # (verified by back-to-back runs).

    f16 = mybir.dt.float16
    f32 = mybir.dt.float32

    nc = bass.Bass("TRN2", target_bir_lowering=False, debug=False)
    # x^T per sample: positions on the contraction (partition) axis
    xd = nc.dram_tensor("xin", [SPC, HWN, C], f16, kind="ExternalInput")
    outs = [
        [
            nc.dram_tensor(f"out{s}_{i}", [128, WIDTHS[i]], f16,
                           kind="ExternalOutput")
            for i in range(4)
        ]
        for s in range(SPC)
    ]

    with TrimTC(nc) as tc:
        with (
            tc.tile_pool(name="const", bufs=1) as const,
            tc.tile_pool(name="gpsum", bufs=2, space="PSUM") as gpsum,
            tc.tile_pool(name="gout", bufs=2) as goutp,
        ):
            xt = []
            for s in range(SPC):
                t0 = const.tile([NCH0, C], f16, name=f"x{s}_0")
                t1 = const.tile([NCH1, C], f16, name=f"x{s}_1")
                xt.append((t0, t1))
            # input DMAs split across the two HWDGE rings: the 128-position
            # chunks (which gate the start matmuls) first
            nc.sync.dma_start(xt[0][0][:], xd[0, 0:NCH0])
            nc.sync.dma_start(xt[1][0][:], xd[1, 0:NCH0])
            nc.scalar.dma_start(xt[0][1][:], xd[0, NCH0:HWN])
            nc.scalar.dma_start(xt[1][1][:], xd[1, NCH0:HWN])

            for s in range(SPC):
                t0, t1 = xt[s]
                pss = []
                # all four start-matmuls back-to-back: they depend only on
                # the 128-chunk; the 68-chunk lands while they run
                for i in range(4):
                    ps = gpsum.tile([128, WIDTHS[i]], f32, name=f"ps{i}",
                                    tag=f"ps{i}")
                    nc.tensor.matmul(
                        ps[:], t0[:, 128 * i : 128 * (i + 1)],
                        t0[:, 128 * i : C], start=True, stop=False,
                    )
                    pss.append(ps)
                for i in range(4):
                    nc.tensor.matmul(
                        pss[i][:], t1[:, 128 * i : 128 * (i + 1)],
                        t1[:, 128 * i : C], start=False, stop=True,
                    )
                    go = goutp.tile([128, WIDTHS[i]], f16, name=f"go{i}",
                                    tag=f"go{i}")
                    # evacuate PSUM on alternating engines so the copies of
                    # consecutive blocks overlap
                    if i % 2 == 0:
                        nc.scalar.copy(go[:], pss[i][:])
                    else:
                        nc.vector.tensor_copy(go[:], pss[i][:])
                    nc.sync.dma_start(outs[s][i][:], go[:])

    _split_multiwaits(nc)
    return nc


def _get_nc():
    if "nc" not in _NC_CACHE:
        _NC_CACHE["nc"] = _build_nc()
    return _NC_CACHE["nc"]


def kernel(x, s1, s2, h1, h2):
    if TRACE:
        _install_ntff_hook()
    from concourse.bass_utils import run_bass_kernel_spmd

    x = np.asarray(x, dtype=np.float32)
    s1 = np.asarray(s1, dtype=np.float64)
    s2 = np.asarray(s2, dtype=np.float64)
    h1 = np.asarray(h1).astype(np.int64)
    h2 = np.asarray(h2).astype(np.int64)

    # [B, C, H, W] -> [B, HW, C] fp16 (positions on the contraction axis)
    xt = np.ascontiguousarray(
        x.reshape(B, C, HWN).transpose(0, 2, 1)
    ).astype(np.float16)

    nc = _get_nc()
    in_maps = [{"xin": xt[SPC * m : SPC * (m + 1)]} for m in range(NCORES)]
    res = run_bass_kernel_spmd(
        nc, in_maps, core_ids=list(range(NCORES)), trace=TRACE
    )
    LAST_RESULT["exec_time_ns"] = res.exec_time_ns
    LAST_RESULT["mean_exec_time_ns"] = res.mean_exec_time_ns
    LAST_RESULT["res"] = res

    # reassemble the symmetric G [B, C, C] from upper-triangle blocks
    G = np.empty((B, C, C), dtype=np.float64)
    for m in range(NCORES):
        for s in range(SPC):
            b = SPC * m + s
            for i in range(4):
                blk = res.results[m][f"out{s}_{i}"].astype(np.float64)
                r = slice(128 * i, 128 * (i + 1))
                G[b, r, 128 * i : C] = blk
                G[b, 128 * i : C, r] = blk.T

    # fixed hash-pair scatter: pair (c1, c2) -> bin (h1[c1]+h2[c2]) mod P
    bins = ((h1[:, None] + h2[None, :]) % PROJ).ravel()
    sw = np.outer(s1, s2).ravel()
    y = np.empty((B, PROJ), dtype=np.float64)
    for b in range(B):
        y[b] = np.bincount(bins, weights=sw * G[b].ravel(), minlength=PROJ)

    y = np.sign(y) * np.sqrt(np.abs(y) + THRESH)
    nrm = np.linalg.norm(y, axis=1, keepdims=True)
    y = y / np.maximum(nrm, L2_EPS)
    return y.astype(np.float32)


# revision 9
# speedup vs baseline: 6.2351x; 1.1847x over previous
"""Compact Bilinear Pooling (count-sketch + FFT + sum-pool) on 8 trn2 cores.

Math: the pooled output of sample b depends on x only through the Gram
matrix  G_b[c1, c2] = sum_n x[b, c1, n] * x[b, c2, n]  (n = spatial pos):

    y_b[k] = sum_{c1, c2} s1[c1] s2[c2] G_b[c1, c2]
                          * [(h1[c1] + h2[c2]) mod 8192 == k]

because the per-position circular convolution of the two count sketches,
summed over positions, is the bilinear form above (expand the sketches:
each channel pair (c1, c2) lands in bin (h1[c1]+h2[c2]) mod P with weight
s1 s2 x[n,c1] x[n,c2]; the position sum yields G_b).  This removes the
FFTs entirely: the device computes the 16 Gram matrices (a [196, 512]^T
@ [196, 512] matmul per sample, 103 MFLOP), and the host applies the
fixed 262144-pair scatter (np.bincount), signed sqrt and L2 normalize.

Sharding: pure data parallel, 2 samples per core.  G is symmetric, so
only the 10 upper-triangle [128, 128] blocks are computed and stored
(rows 128i, cols 128i..512, widths 512/384/256/128).  Contraction over
positions is split 196 = 128 + 68; the four start-matmuls of a sample
run back-to-back so only the first 128-position chunk gates the PE.
PSUM is evacuated alternately by ScalarE and VectorE (fp32 -> fp16) and
each block DMAs out as soon as it is ready, smallest block last to
minimize the completion tail.  ~1 MB HBM traffic per core: memory-bound.
"""

import numpy as np

PROJ = 8192
B, C, H, W = 16, 512, 14, 14
HWN = H * W           # 196 positions per sample
NCORES = 8
SPC = B // NCORES     # 2 samples per core
NCH0 = 128            # position-chunk sizes (contraction dim)
NCH1 = HWN - NCH0     # 68
WIDTHS = [C - 128 * i for i in range(4)]  # triangle block widths
THRESH = 1e-8
L2_EPS = 1e-12

TRACE = False         # set by test.py to collect HW timing
LAST_RESULT = {}      # exec_time_ns etc. for test.py

_NC_CACHE = {}


def _install_ntff_hook():
    """The container's antenv stub lacks axon_hooks, so the boot-time NTFF
    profile hook install silently degraded.  Recreate it: a tiny module
    backed by ctypes calls into libaxon_pjrt.so (same mechanism as
    trn_agent_boot.trn_boot)."""
    import sys, types
    if "antenv.axon_hooks" in sys.modules:
        return
    try:
        from trn_agent_boot.trn_boot import _ntff_profile_via_ctypes
        hook = _ntff_profile_via_ctypes("/opt/axon/libaxon_pjrt.so")
    except Exception:
        hook = None
    mod = types.ModuleType("antenv.axon_hooks")
    _state = {"hook": hook}
    mod.get_axon_ntff_profile_hook = lambda: _state["hook"]
    mod.set_axon_ntff_profile_hook = lambda h: _state.__setitem__("hook", h)
    sys.modules["antenv.axon_hooks"] = mod
    try:
        import antenv
        antenv.axon_hooks = mod
    except Exception:
        pass


def _split_multiwaits(nc, maxw=1):
    """This container's walrus codegen rejects instructions carrying more
    than one sem wait ("Too many sync wait commands").  Hoist excess waits
    onto same-engine NoOps inserted immediately before the instruction —
    semantically identical (the engine sequencer blocks either way)."""
    import bass_rust
    import concourse.mybir as mybir

    for f in nc.m.functions:
        for bb in f.blocks:
            il = bb.instructions
            new = []
            changed = False
            for inst in il:
                si = inst.sync_info
                waits = list(si.on_wait) if si is not None else []
                if len(waits) > maxw:
                    keep = waits[-maxw:]
                    for w in waits[:-maxw]:
                        nop = mybir.InstNoOp(
                            name=nc.get_next_instruction_name(),
                            engine=inst.engine,
                            sync_info=bass_rust.SyncInfo(
                                on_wait=[w], on_update=[]
                            ),
                            bass_nofuse=True,
                        )
                        nc.register_instruction(nop)
                        new.append(nop)
                    inst.sync_info = bass_rust.SyncInfo(
                        on_wait=keep, on_update=list(si.on_update)
                    )
                    changed = True
                new.append(inst)
            if changed:
                bb.instructions = new


def _build_nc():
    import concourse.bass as bass
    import concourse.mybir as mybir
    import concourse.tile as tile
    from concourse.vector_clock import ScopedClock

    class TrimTC(tile.TileContext):
        # Stock tail: drain + barrier + sem clears + barrier (~10us).
        # The sem clears are required for NEFF re-execution, but they can
        # ride behind the first barrier without a trailing second barrier:
        # nothing after them reads the sems, and the next execution's
        # preamble re-syncs the engines.
        def _drain_and_barrier(self, tick_clock, wait_clock):
            drain_inst = self.nc.sync.drain()
            wait_clock.add_sem_waits(
                drain_inst.ins, ScopedClock({None: tick_clock.global_clock})
            )
            popped = self.nc._tile_sem_poison_stack.pop()
            assert popped is self._sem_poison
            # no barrier / sem clears: the SP drain already waits on the
            # final DMA sems, NRT's own completion protocol syncs engines,
            # and the execution preamble re-initializes semaphores
            # (verified by back-to-back runs).

    bf16 = mybir.dt.bfloat16
    f16 = mybir.dt.float16
    f32 = mybir.dt.float32
    TRIW = sum(WIDTHS)  # 1280 packed triangle columns

    nc = bass.Bass("TRN2", target_bir_lowering=False, debug=False)
    # x^T per sample: positions on the contraction (partition) axis
    xd = nc.dram_tensor("xin", [SPC, HWN, C], f16, kind="ExternalInput")
    # packed upper-triangle blocks, [128, 512|384|256|128] side by side
    out_d = nc.dram_tensor("out", [SPC, 128, TRIW], f16, kind="ExternalOutput")

    with TrimTC(nc) as tc:
        with (
            tc.tile_pool(name="const", bufs=1) as const,
            tc.tile_pool(name="gpsum", bufs=2, space="PSUM") as gpsum,
            tc.tile_pool(name="gpsum1", bufs=1, space="PSUM") as gpsum1,
            tc.tile_pool(name="gout", bufs=2) as goutp,
        ):
            xt = []
            for s in range(SPC):
                t0 = const.tile([NCH0, C], f16, name=f"x{s}_0")
                t1 = const.tile([NCH1, C], f16, name=f"x{s}_1")
                xt.append((t0, t1))
            # input DMAs split across the two HWDGE rings: the 128-position
            # chunks (which gate the start matmuls) first
            nc.sync.dma_start(xt[0][0][:], xd[0, 0:NCH0])
            nc.sync.dma_start(xt[1][0][:], xd[1, 0:NCH0])
            nc.scalar.dma_start(xt[0][1][:], xd[0, NCH0:HWN])
            nc.scalar.dma_start(xt[1][1][:], xd[1, NCH0:HWN])

            # Warm the PE clock gate (HAM) with throwaway matmuls on
            # never-written SBUF garbage while the input DMAs land.
            wsrc = const.tile([128, 128], bf16, name="warm_src")
            wrhs = const.tile([128, 64], bf16, name="warm_rhs")
            nc.gpsimd.memset(wsrc[:], 0.0)
            nc.gpsimd.memset(wrhs[:], 0.0)
            wps = gpsum1.tile([128, 64], f32, name="wps", tag="wps")
            for _ in range(44):
                nc.tensor.matmul(wps[:], wsrc[:], wrhs[:], start=True, stop=True)

            for s in range(SPC):
                t0, t1 = xt[s]
                go = goutp.tile([128, TRIW], f16, name=f"go{s}", tag="go")
                pss = []
                # all four start-matmuls back-to-back: they depend only on
                # the 128-chunk; the 68-chunk lands while they run
                for i in range(4):
                    # ps0 single-buffered (PSUM is 8 banks): sample 1's
                    # first matmul waits on sample 0's ps0 evacuation,
                    # which is long done by then
                    pool = gpsum1 if i == 0 else gpsum
                    ps = pool.tile([128, WIDTHS[i]], f32, name=f"ps{i}",
                                   tag=f"ps{i}")
                    nc.tensor.matmul(
                        ps[:], t0[:, 128 * i : 128 * (i + 1)],
                        t0[:, 128 * i : C], start=True, stop=False,
                    )
                    pss.append(ps)
                off = 0
                for i in range(4):
                    nc.tensor.matmul(
                        pss[i][:], t1[:, 128 * i : 128 * (i + 1)],
                        t1[:, 128 * i : C], start=False, stop=True,
                    )
                    # evacuate PSUM on alternating engines so the copies of
                    # consecutive blocks overlap
                    dst = go[:, off : off + WIDTHS[i]]
                    if i % 2 == 0:
                        nc.scalar.copy(dst, pss[i][:])
                    else:
                        nc.vector.tensor_copy(dst, pss[i][:])
                    off += WIDTHS[i]
                # one 320 KB store per sample on the otherwise-idle SP ring
                nc.sync.dma_start(out_d[s], go[:])

    _split_multiwaits(nc)
    return nc


def _get_nc():
    if "nc" not in _NC_CACHE:
        _NC_CACHE["nc"] = _build_nc()
    return _NC_CACHE["nc"]


def kernel(x, s1, s2, h1, h2):
    if TRACE:
        _install_ntff_hook()
    from concourse.bass_utils import run_bass_kernel_spmd

    x = np.asarray(x, dtype=np.float32)
    s1 = np.asarray(s1, dtype=np.float64)
    s2 = np.asarray(s2, dtype=np.float64)
    h1 = np.asarray(h1).astype(np.int64)
    h2 = np.asarray(h2).astype(np.int64)

    # [B, C, H, W] -> [B, HW, C] fp16 (positions on the contraction axis)
    xt = np.ascontiguousarray(
        x.reshape(B, C, HWN).transpose(0, 2, 1)
    ).astype(np.float16)

    nc = _get_nc()
    in_maps = [{"xin": xt[SPC * m : SPC * (m + 1)]} for m in range(NCORES)]
    res = run_bass_kernel_spmd(
        nc, in_maps, core_ids=list(range(NCORES)), trace=TRACE
    )
    LAST_RESULT["exec_time_ns"] = res.exec_time_ns
    LAST_RESULT["mean_exec_time_ns"] = res.mean_exec_time_ns
    LAST_RESULT["res"] = res

    # reassemble the symmetric G [B, C, C] from packed upper-triangle blocks
    G = np.empty((B, C, C), dtype=np.float64)
    for m in range(NCORES):
        o = res.results[m]["out"].astype(np.float64)  # [SPC, 128, 1280]
        for s in range(SPC):
            b = SPC * m + s
            off = 0
            for i in range(4):
                blk = o[s, :, off : off + WIDTHS[i]]
                off += WIDTHS[i]
                r = slice(128 * i, 128 * (i + 1))
                G[b, r, 128 * i : C] = blk
                G[b, 128 * i : C, r] = blk.T

    # fixed hash-pair scatter: pair (c1, c2) -> bin (h1[c1]+h2[c2]) mod P
    bins = ((h1[:, None] + h2[None, :]) % PROJ).ravel()
    sw = np.outer(s1, s2).ravel()
    y = np.empty((B, PROJ), dtype=np.float64)
    for b in range(B):
        y[b] = np.bincount(bins, weights=sw * G[b].ravel(), minlength=PROJ)

    y = np.sign(y) * np.sqrt(np.abs(y) + THRESH)
    nrm = np.linalg.norm(y, axis=1, keepdims=True)
    y = y / np.maximum(nrm, L2_EPS)
    return y.astype(np.float32)
